# revision 30
# baseline (speedup 1.0000x reference)
"""Trainium2 Bass kernel for nn_ALLonBert_v3 (segment_reduce + tiny classifier).

Computation (per batch row b):
  means[k, :] = mean of sequence_outputs[b, t, :] over tokens t in segment k
  logits[b, k, c] = means[k, :] @ W[c, :] + b[c]

Device strategy (pure data-parallel, 8 batch rows per core, no collectives):
  - Host builds a one-hot assignment matrix A[t, k] (token t belongs to
    segment k), mirroring the host-side SEP scan the original module performs.
    One-hot values are exact in bf16; the 1/cnt mean scaling is applied to the
    final logits via a per-partition tensor_scalar (invcnt input, f32).
  - x is DMA-cast f32 -> bf16 in flight (SWDGE), halving TensorEngine
    streaming time; segment sums accumulate in f32 PSUM:
      sums[32p:32p+32, h] += A_pair[t, :].T @ x[t, h]  over 4 token chunks
  - DVE: PSUM -> SBUF copies, then logits = (means * Wb).sum(h) * invcnt.
"""

import sys

for _p in ("/opt/trn_rl_repo", "/opt/pypackages"):
    if _p not in sys.path:
        sys.path.insert(0, _p)

import ml_dtypes
import numpy as np

import concourse.bacc as bacc
import concourse.mybir as mybir
import concourse.tile as tile
from concourse.bass_utils import run_bass_kernel_spmd

B, S, H, NSEG = 64, 512, 768, 16
NCORES = 8
RPC = B // NCORES       # batch rows per core = 8
P = 128                 # partitions
NCH = S // P            # token chunks per row = 4
ROWS_PER_DMA = 1        # batch rows loaded per DMA instruction

F32 = mybir.dt.float32
BF16 = mybir.dt.bfloat16
NPBF16 = ml_dtypes.bfloat16

_graph_cache = {}


def _build_graph(rows_per_dma=ROWS_PER_DMA, xbufs=6, reps=1):
    nc = bacc.Bacc("TRN2", target_bir_lowering=False, debug=False,
                   num_devices=NCORES)

    x_ext = nc.declare_dram_parameter("x", [RPC * S, H], F32, isOutput=False)
    a_ext = nc.declare_dram_parameter("apack", [P, RPC * NCH * 2 * NSEG], BF16,
                                      isOutput=False)
    w_ext = nc.declare_dram_parameter("wflat", [1, 2 * H], F32, isOutput=False)
    ic_ext = nc.declare_dram_parameter("invcnt", [P, 1], F32, isOutput=False)
    out_ext = nc.declare_dram_parameter("out", [P, 2], F32, isOutput=True)

    n_groups = RPC // rows_per_dma
    # x[(g rg c p), h] -> [g][p][rg c h]: group g, row-in-group rg, chunk c.
    xv = x_ext.ap().rearrange("(g rg c p) h -> g p rg c h",
                              g=n_groups, rg=rows_per_dma, c=NCH, p=P)

    with tile.TileContext(nc) as tc:
        with (
            tc.tile_pool(name="consts", bufs=1) as consts,
            tc.tile_pool(name="xp", bufs=xbufs) as xp,
            tc.tile_pool(name="ps1", bufs=3, space="PSUM") as ps1,
            tc.tile_pool(name="ps2", bufs=3, space="PSUM") as ps2,
            tc.tile_pool(name="tmp", bufs=2) as tmpp,
        ):
            a_sb = consts.tile([P, RPC * NCH * 2 * NSEG], BF16)
            nc.sync.dma_start(out=a_sb[:], in_=a_ext.ap())
            # Classifier constants: W rows broadcast across partitions on
            # gpsimd, invcnt per-partition scalars (tiny DMAs).
            ic_sb = consts.tile([P, 1], F32)
            nc.sync.dma_start(out=ic_sb[:], in_=ic_ext.ap())
            w_sb = consts.tile([1, 2 * H], F32)
            nc.sync.dma_start(out=w_sb[:], in_=w_ext.ap())
            wb_sb = consts.tile([P, 2 * H], F32)
            nc.gpsimd.partition_broadcast(wb_sb[:], w_sb[:])

            # Rows are processed in pairs: the lhsT for a (row, chunk) matmul
            # is [128, 32] with the row's one-hot columns in its half (other
            # half zero), so PSUM tiles are [32, *] and all partition bases
            # stay 32-aligned (a BIR verifier requirement).
            # x tiles arrive in groups of `rows_per_dma` rows; per-chunk
            # (tile, offset) for each row is tracked so pairing is independent
            # of DMA granularity. The first `split_rows` rows are loaded with
            # one DMA per token-chunk (separate tiles) so the first matmuls
            # start ~3us earlier; later rows use one large DMA per group.
            # reps>1 repeats the whole pipeline (HW timing harness only).
            for rep in range(reps):
              split_rows = min(2, rows_per_dma) if rep == 0 else 0
              row_src = {}
              logits_sb = tmpp.tile([P, 2], F32, tag="logits")
              for g in range(n_groups):
                if g * rows_per_dma < split_rows:
                    for rg in range(rows_per_dma):
                        r = g * rows_per_dma + rg
                        srcs = []
                        for c in range(NCH):
                            xc = xp.tile([P, H], BF16, tag="xchunk")
                            nc.gpsimd.dma_start(out=xc[:],
                                                in_=xv[g][:, rg, c, :])
                            srcs.append((xc, 0))
                        row_src[r] = srcs
                else:
                    xt = xp.tile([P, rows_per_dma * NCH * H], BF16)
                    nc.gpsimd.dma_start(   # SWDGE: casts f32 -> bf16 in flight
                        out=xt[:].rearrange("p (rg c h) -> p rg c h",
                                            rg=rows_per_dma, c=NCH),
                        in_=xv[g],
                    )
                    for rg in range(rows_per_dma):
                        row_src[g * rows_per_dma + rg] = [
                            (xt, (rg * NCH + c) * H) for c in range(NCH)]
                while True:
                    pairs = [r // 2 for r in row_src
                             if r % 2 == 0 and r + 1 in row_src]
                    if not pairs:
                        break
                    pair = min(pairs)
                    m1 = ps1.tile([2 * NSEG, 512], F32)
                    m2 = ps2.tile([2 * NSEG, 256], F32)
                    for half in range(2):
                        r = 2 * pair + half
                        srcs = row_src.pop(r)
                        for c in range(NCH):
                            lhs = a_sb[:, (r * NCH + c) * 2 * NSEG:
                                       (r * NCH + c + 1) * 2 * NSEG]
                            xt_r, xoff = srcs[c]
                            first = (half == 0 and c == 0)
                            last = (half == 1 and c == NCH - 1)
                            nc.tensor.matmul(m1[:], lhs,
                                             xt_r[:, xoff:xoff + 512],
                                             start=first, stop=last)
                            nc.tensor.matmul(m2[:], lhs,
                                             xt_r[:, xoff + 512:xoff + H],
                                             start=first, stop=last)
                    # Classifier for this pair, reading segment sums straight
                    # from PSUM (no PSUM->SBUF means copy on the tail):
                    # logits[k', c] = invcnt[k'] * sum_h sums[k', h] W[c, h]
                    # (tensor_tensor_reduce crashes the device in this
                    # runtime; use separate multiply + reduce DVE ops.)
                    po = 2 * NSEG * pair
                    wbp = wb_sb[po:po + 2 * NSEG, :]
                    for cc in range(2):
                        pr1 = tmpp.tile([2 * NSEG, 512], F32, tag="pr1")
                        nc.vector.tensor_tensor(
                            out=pr1[:], in0=m1[:],
                            in1=wbp[:, cc * H:cc * H + 512],
                            op=mybir.AluOpType.mult)
                        pr2 = tmpp.tile([2 * NSEG, 256], F32, tag="pr2")
                        nc.vector.tensor_tensor(
                            out=pr2[:], in0=m2[:],
                            in1=wbp[:, cc * H + 512:(cc + 1) * H],
                            op=mybir.AluOpType.mult)
                        r1 = tmpp.tile([2 * NSEG, 1], F32, tag="r1")
                        nc.vector.tensor_reduce(
                            out=r1[:], in_=pr1[:],
                            axis=mybir.AxisListType.X, op=mybir.AluOpType.add)
                        r2 = tmpp.tile([2 * NSEG, 1], F32, tag="r2")
                        nc.vector.tensor_reduce(
                            out=r2[:], in_=pr2[:],
                            axis=mybir.AxisListType.X, op=mybir.AluOpType.add)
                        nc.vector.tensor_add(
                            out=logits_sb[po:po + 2 * NSEG, cc:cc + 1],
                            in0=r1[:], in1=r2[:])

              nc.vector.tensor_scalar_mul(logits_sb[:], logits_sb[:],
                                          ic_sb[:, 0:1])
              nc.sync.dma_start(out=out_ext.ap(), in_=logits_sb[:])

    nc.compile()
    return nc


def _get_graph(rows_per_dma=ROWS_PER_DMA, xbufs=6, reps=1):
    key = (rows_per_dma, xbufs, reps)
    if key not in _graph_cache:
        _graph_cache[key] = _build_graph(rows_per_dma, xbufs, reps)
    return _graph_cache[key]


def _segment_onehot(sep_positions: np.ndarray):
    """One-hot A[b, t, k] (reference semantics) and counts [b, k]."""
    t = np.arange(S)
    sep = np.asarray(sep_positions)
    seg_id = (t[None, None, :] >= sep[:, :, None]).sum(axis=1)        # [B, S]
    is_sep = (t[None, None, :] == sep[:, :, None]).any(axis=1)        # [B, S]
    valid = (t[None, :] >= 1) & (~is_sep) & (seg_id < NSEG)
    seg_clipped = np.where(valid, seg_id, NSEG)
    a = (seg_clipped[:, :, None] == np.arange(NSEG)[None, None, :])
    a = a.astype(np.float32)                                          # [B, S, NSEG]
    cnts = a.sum(axis=1)                                              # [B, NSEG]
    return a, cnts


def make_in_maps(sequence_outputs, sep_positions, W):
    x = np.ascontiguousarray(sequence_outputs, dtype=np.float32)
    w = np.ascontiguousarray(W, dtype=np.float32)
    a_onehot, cnts = _segment_onehot(sep_positions)
    inv = (1.0 / np.maximum(cnts, 1.0)).astype(np.float32)            # [B, NSEG]

    wflat = w.reshape(1, 2 * H)
    in_maps = []
    for m in range(NCORES):
        rows = slice(m * RPC, (m + 1) * RPC)
        xs = x[rows].reshape(RPC * S, H)
        # Paired layout: per (row, chunk) a [P, 32] block whose half
        # (row parity) holds the one-hot columns, the other half zeros.
        ach = a_onehot[rows].reshape(RPC, NCH, P, NSEG)     # [r, c, p, k]
        apad = np.zeros((RPC, NCH, P, 2 * NSEG), dtype=np.float32)
        for r in range(RPC):
            off = (r % 2) * NSEG
            apad[r, :, :, off:off + NSEG] = ach[r]
        ash = apad.transpose(2, 0, 1, 3).reshape(P, RPC * NCH * 2 * NSEG)
        # invcnt partition layout: partition 32*pair + 16*half + k
        icv = inv[rows].reshape(RPC // 2, 2, NSEG).reshape(P, 1)
        in_maps.append({
            "x": np.ascontiguousarray(xs),
            "apack": np.ascontiguousarray(ash.astype(NPBF16)),
            "wflat": wflat,
            "invcnt": np.ascontiguousarray(icv),
        })
    return in_maps


def kernel(sequence_outputs, sep_positions, W, b):
    bias = np.asarray(b, dtype=np.float32)
    in_maps = make_in_maps(sequence_outputs, sep_positions, W)
    nc = _get_graph()
    res = run_bass_kernel_spmd(nc, in_maps, core_ids=list(range(NCORES)))
    out = np.concatenate(
        [res.results[m]["out"].reshape(RPC, NSEG, 2) for m in range(NCORES)],
        axis=0,
    )
    return out + bias[None, None, :]
